# revision 18
# baseline (speedup 1.0000x reference)
"""Trainium2 Bass kernel for causal multi-head attention (B=4,S=2048,D=1024,N=16,H=64).

Sharding: 8 cores = (batch b in 0..3) x (head-group g in 0..1). Each core gets
residual[b] and 8 heads' worth of W_Q/K/V/O, computes the partial output
sum_{n in group} attn_n @ W_O[n]  ->  [2048,1024]; host adds the two
head-group partials per batch. No collectives needed.

v3 structure (per core, all-f16 matmuls):
  - Inputs cast fp32->fp16 during the DMA itself (SWDGE gpsimd loads), ALL
    kicked back-to-back at program start so the ring streams the 16MB of
    reads gap-free. All transposes (W^T and X^T) are f16 PE transposes +
    DVE evacs: XBAR DMA-transpose consumers made the tile scheduler
    (which models DMA as one serial resource) interleave + semaphore-
    serialize the loads, doubling phase-1 time.
  - A ~30-matmul PE warm-up spin at t=0 keeps the HAM clock-gate at 8/8
    (otherwise the load-gated prologue runs the PE at 1.2GHz).
  - Pair 0 streams in ASCENDING G order: per-G prelude transposes x group
    G and runs the projections it unblocks, so attention starts as soon as
    the first x chunk lands instead of after the full load.
  - Scores S^T = K^T.T @ Q^T, PAIRED: two k-tiles' scores land in one
    2-bank PSUM tile and ONE Exp instruction covers both ([128, 2, 512-o]),
    halving ACT per-instruction overhead (the dominant HW win, ~100us).
    Each sub-tile's matmul computes only its true causal range; the Exp
    reads the widened AP (stale PSUM on the odd tile is never consumed).
  - PV with a ones column per head (softmax denominator on PE for free).
  - Normalize on DVE, transpose AO on PE, O-projection f16 pulled into
    pair 3's exp-gated attention gaps as generators (a solid O-proj block
    left the attention windows at ~50% PE duty, tripping the HAM activity
    throttle to half clock for ~24us).
  - Inputs arrive as ONE dram blob (x + 4 W's): fewer per-call PJRT args
    cut the host dispatch cost of each execution ~460us -> ~300us.
  fp8 variants (DoubleRow proj/PV/scores) are implemented behind env flags
  but default OFF: plain fp8 evac+matmul produces NaN on this HW stack
  despite passing CoreSim and isolated-primitive HW probes; fp8 proj/PV
  also exceed the 2e-2 error budget (4.7e-2 / 2.5e-2).
"""

import sys

sys.path.insert(0, "/opt/trn_rl_repo")

import numpy as np
import concourse.bass as bass
import concourse.mybir as mybir
import concourse.tile as tile
from concourse.bass_utils import run_bass_kernel_spmd
from concourse.masks import make_identity

F32 = mybir.dt.float32
F32R = mybir.dt.float32r
F16 = mybir.dt.float16
F8 = mybir.dt.float8e4
AF = mybir.ActivationFunctionType
DR = mybir.MatmulPerfMode.DoubleRow

S = 2048
D = 1024
NH = 8  # heads per core
H = 64
P = 128
ST = S // P  # 16
DT = D // P  # 8
NPAIR = NH // 2  # 4
SCALE = 1.0 / 8.0  # 1/sqrt(H)
COMPUTE_MAX_WAITS = 1

import os

QKV_FP8 = os.environ.get("K_QKV_FP8", "0") == "1"  # fp8 DoubleRow projections
PV_FP8 = os.environ.get("K_PV_FP8", "0") == "1"  # fp8 P^T + fp8 V, DoubleRow PV
PH1 = os.environ.get("K_PH1", "xbar")  # "xbar" | "pe" phase-1 scheme
EXPP = os.environ.get("K_EXPP", "1") == "1"  # paired 2-bank Exp
SC_MODE = os.environ.get("K_SC", "f16")  # f16 | plain8 | dr
SC_FP8 = SC_MODE == "dr"  # fp8 DoubleRow scores (fold + DR)
SC_P8 = SC_MODE in ("plain8", "dr")  # fp8 qt/kt evac

XW_DT = F8 if QKV_FP8 else F16
PD = F8 if PV_FP8 else F16  # pattern dtype

CTRL_INSTS = ("InstDrain", "InstNop", "InstEventSemaphoreOp", "InstSemaphoreOp")


def split_excess_waits(nc, max_waits=1, compute_max_waits=1):
    """This walrus build rejects >1 sync wait on CTRL-class instructions
    (Drain/NoOp). Move excess waits onto same-engine NOPs inserted
    immediately before. Compute instructions may keep compute_max_waits."""
    n_split = 0
    for bb in nc.main_func.blocks:
        insts = list(bb.instructions)
        out = []
        for ins in insts:
            si = ins.sync_info
            lim = max_waits if type(ins).__name__ in CTRL_INSTS else compute_max_waits
            if si is not None and si.on_wait and len(si.on_wait) > lim:
                waits = list(si.on_wait)
                while len(waits) > lim:
                    chunk, waits = waits[:1], waits[1:]
                    nop = nc.engines[ins.engine].nop(nofuse=True).ins
                    for b2 in nc.main_func.blocks:
                        if nop in b2.instructions:
                            b2.instructions.remove(nop)
                            break
                    if nop.sync_info is None:
                        nop.sync_info = mybir.SyncInfo(on_wait=[], on_update=[])
                    nop.sync_info.on_wait = chunk
                    out.append(nop)
                    n_split += 1
                si.on_wait = waits
            out.append(ins)
        bb.instructions[:] = out
    return n_split


def work_tile(pool, tag, shape, dtype, bufs):
    return pool.tile(shape, dtype, tag=tag, bufs=bufs, name=tag)


def emit(nc, tc, x, wq_d, wk_d, wv_d, wo_d, out_d, dbg=None):
    with (
        tc.tile_pool(name="const", bufs=1) as constp,
        tc.tile_pool(name="persist", bufs=1) as persist,
        tc.tile_pool(name="psM", bufs=2, space="PSUM") as psM,
    ):
        ident = constp.tile([P, P], F16)

        xt = persist.tile([P, DT * S], XW_DT)
        wqt = persist.tile([P, DT * 512], XW_DT)
        wkt = persist.tile([P, DT * 512], XW_DT)
        wvt = persist.tile([P, DT * 512], XW_DT)
        QK_DT = F8 if SC_P8 else F16
        qt = persist.tile([P, NPAIR * S], QK_DT)
        kt = persist.tile([P, NPAIR * S], QK_DT)
        qfv = kfv = None
        if SC_FP8:
            # folded layouts for DoubleRow scores: [32-part head strips x
            # 2 k-subtiles]; head-half hh at partition base 64*hh, pair t in
            # column block t. h-dim index = 32*i + p within each strip.
            qf = persist.tile([P, NPAIR * 2 * S], F8)
            kf = persist.tile([P, NPAIR * 2 * S], F8)
            qfv = qf.rearrange("p (b i s) -> p b i s", b=NPAIR, i=2)
            kfv = kf.rearrange("p (b i s) -> p b i s", b=NPAIR, i=2)

        v4 = persist.tile([P, ST * NH * 65], PD)
        aot = persist.tile([P, NPAIR * S], F16)
        wo_sb = persist.tile([P, NPAIR * D], F16)

        v4v = v4.rearrange("p (i h e) -> p i h e", i=ST, h=NH)
        xtv = xt.rearrange("p (k s) -> p k s", k=DT)
        wqtv = wqt.rearrange("p (k h) -> p k h", k=DT)
        wktv = wkt.rearrange("p (k h) -> p k h", k=DT)
        wvtv = wvt.rearrange("p (k h) -> p k h", k=DT)

        # projection-chain generators: yield after each matmul so the pair
        # loop can interleave single proj matmuls into ACT-bound gaps
        def gen_qk(wtv_src, dst, c, t, fold_dstv=None):
            pq = psM.tile([P, 512], F32, tag="proj")
            if QKV_FP8:
                for k2 in range(DT // 2):
                    nc.tensor.matmul(
                        pq,
                        lhsT=wtv_src[:, 2 * k2 : 2 * k2 + 2, t * P : (t + 1) * P],
                        rhs=xtv[:, 2 * k2 : 2 * k2 + 2, c * 512 : (c + 1) * 512],
                        start=(k2 == 0),
                        stop=(k2 == DT // 2 - 1),
                        perf_mode=DR,
                    )
                    yield
            else:
                for k in range(DT):
                    nc.tensor.matmul(
                        pq,
                        lhsT=wtv_src[:, k, t * P : (t + 1) * P],
                        rhs=xtv[:, k, c * 512 : (c + 1) * 512],
                        start=(k == 0),
                        stop=(k == DT - 1),
                    )
                    yield
            nc.vector.tensor_copy(
                dst[:, t * S + c * 512 : t * S + (c + 1) * 512], pq
            )
            if fold_dstv is not None:
                # partition-fold this chunk [64 h, 512] -> [32, 2, 512]
                for hh in range(2):
                    for i2 in range(2):
                        nc.sync.dma_start(
                            out=fold_dstv[
                                64 * hh : 64 * hh + 32,
                                t,
                                i2,
                                c * 512 : (c + 1) * 512,
                            ],
                            in_=dst[
                                64 * hh + 32 * i2 : 64 * hh + 32 * i2 + 32,
                                t * S + c * 512 : t * S + (c + 1) * 512,
                            ],
                        )

        def gen_v(i, t):
            pv = psM.tile([P, 512], F32, tag="proj")
            if QKV_FP8:
                for k2 in range(DT // 2):
                    nc.tensor.matmul(
                        pv[:, 0:P],
                        lhsT=xtv[:, 2 * k2 : 2 * k2 + 2, i * P : (i + 1) * P],
                        rhs=wvtv[:, 2 * k2 : 2 * k2 + 2, t * P : (t + 1) * P],
                        start=(k2 == 0),
                        stop=(k2 == DT // 2 - 1),
                        perf_mode=DR,
                    )
                    yield
            else:
                for k in range(DT):
                    nc.tensor.matmul(
                        pv[:, 0:P],
                        lhsT=xtv[:, k, i * P : (i + 1) * P],
                        rhs=wvtv[:, k, t * P : (t + 1) * P],
                        start=(k == 0),
                        stop=(k == DT - 1),
                    )
                    yield
            nc.vector.tensor_copy(
                v4v[:, i, 2 * t : 2 * t + 2, 0:64],
                pv[:, 0:P].rearrange("p (h e) -> p h e", h=2),
            )

        def gen_oproj(i):
            # O-projection for row tile i: out[i] = sum_t aot[t,i]^T @ wo[t];
            # both 512-col chunks batched into one 512KB store
            osb = work_tile(persist, "osb", [P, D], F32, 2)
            for c in range(2):
                oo = psM.tile([P, 512], F32, tag="proj")
                for tp in range(NPAIR):
                    nc.tensor.matmul(
                        oo,
                        lhsT=aot[:, tp * S + i * P : tp * S + (i + 1) * P],
                        rhs=wo_sb[:, tp * D + c * 512 : tp * D + (c + 1) * 512],
                        start=(tp == 0),
                        stop=(tp == NPAIR - 1),
                    )
                    yield
                nc.vector.tensor_copy(osb[:, c * 512 : (c + 1) * 512], oo)
            nc.sync.dma_start(out=out_d[i * P : (i + 1) * P, :], in_=osb)

        def em_qk(wtv_src, dst, c, t, fold_dstv=None):
            for _ in gen_qk(wtv_src, dst, c, t, fold_dstv):
                pass

        def em_v(i, t):
            for _ in gen_v(i, t):
                pass

        # ---- Phase 1: SWDGE cast-loads (fp32->fp16 in DMA), all kicked
        # back-to-back at program start. ALL transposes (W^T and X^T) are f16
        # PE transposes: with only PE/DVE consumers the tile scheduler leaves
        # the SWDGE kicks wait-free, so the ring streams the 16MB of input
        # reads gap-free (XBAR transposes made the scheduler interleave +
        # semaphore-serialize the loads, stretching phase 1 2x). A PE warm-up
        # spin covers the first ~8us so the HAM clock-gate is at 8/8 before
        # real compute starts (otherwise the whole prologue runs at 1.2GHz).
        stage = tc.alloc_tile_pool(name="stage", bufs=1)
        # PE warm-up spin: ~30 512-col matmuls on a zeroed f16 tile; the
        # products land in a scratch PSUM bank nobody reads
        spin_src = constp.tile([P, 512], F16)
        nc.vector.memset(spin_src, 0.0)
        spin_ps = psM.tile([P, 1024], F32, tag="st2")
        for _ in range(30):
            nc.tensor.matmul(
                spin_ps[:, 0:512],
                lhsT=spin_src[:, 0:P],
                rhs=spin_src,
                start=True,
                stop=True,
            )
        # loads in consumption order: x group 0 first (it gates the whole
        # pair-0 stream), then W_Q/K/V, then x groups 1-3, then W_O.
        xf_by_g = {}

        def xload(ig):
            xf = stage.tile([P, 4 * D], F16, tag="xf", bufs=4)
            nc.gpsimd.dma_start(
                out=xf,
                in_=x[ig * 4 * P : (ig + 1) * 4 * P, :].rearrange(
                    "(a p) d -> p a d", p=P
                ),
            )
            xf_by_g[ig] = xf.rearrange("p (a d) -> p a d", a=4)

        xload(0)
        wq16 = stage.tile([P, 4 * D], F16, tag="w16", bufs=3)
        nc.gpsimd.dma_start(out=wq16, in_=wq_d.rearrange("(a p) d -> p a d", p=P))
        wk16 = stage.tile([P, 4 * D], F16, tag="w16", bufs=3)
        nc.gpsimd.dma_start(out=wk16, in_=wk_d.rearrange("(a p) d -> p a d", p=P))
        wv16 = stage.tile([P, 4 * D], F16, tag="w16", bufs=3)
        nc.gpsimd.dma_start(out=wv16, in_=wv_d.rearrange("(a p) d -> p a d", p=P))
        for ig in range(1, 4):
            xload(ig)
        # W_O: cast-load directly into its final SBUF layout
        nc.gpsimd.dma_start(
            out=wo_sb.rearrange("p (a d) -> p a d", a=NPAIR),
            in_=wo_d.rearrange("(a p) d -> p a d", p=P),
        )

        # gpsimd-engine setup, after all the DMA kicks
        make_identity(nc, ident)
        nc.gpsimd.memset(v4v[:, :, :, 64], 1.0)

        def x_unit(ig, ii):
            # f16 PE transpose of [128 s, 1024 d] + DVE evac
            st8 = 4 * ig + ii
            pt = psM.tile([P, 1024], F16, tag="st2")
            for k in range(DT):
                nc.tensor.transpose(
                    pt[:, k * P : (k + 1) * P],
                    xf_by_g[ig][:, ii, k * P : (k + 1) * P],
                    ident,
                )
            nc.vector.tensor_copy(
                xtv[:, 0:DT, st8 * P : (st8 + 1) * P],
                pt.rearrange("p (k c) -> p k c", k=DT),
            )

        def w_unit(w16v, wtv_dst, j):
            # f16 PE transpose [128 h, 1024 d] -> [128 d, 8k, 128 h]
            pt = psM.tile([P, 1024], F16, tag="st2")
            for k in range(DT):
                nc.tensor.transpose(
                    pt[:, k * P : (k + 1) * P],
                    w16v[:, j, k * P : (k + 1) * P],
                    ident,
                )
            nc.vector.tensor_copy(
                wtv_dst[:, 0:DT, j * P : (j + 1) * P],
                pt.rearrange("p (k c) -> p k c", k=DT),
            )

        wq16v = wq16.rearrange("p (a d) -> p a d", a=4)
        wk16v = wk16.rearrange("p (a d) -> p a d", a=4)
        wv16v = wv16.rearrange("p (a d) -> p a d", a=4)

        # PE order = data-arrival order: x0 transposes, then W^T (j=0 blocks
        # first: they alone gate pair-0's projections)
        for ii in range(4):
            x_unit(0, ii)
        for j in range(4):
            w_unit(wq16v, wqtv, j)
        for j in range(4):
            w_unit(wk16v, wktv, j)
        for j in range(4):
            w_unit(wv16v, wvtv, j)

        def prelude(G):
            # pair-0 per-G streaming prelude: transpose x group G, then the
            # projections it unblocks (Q/K chunk c=G, V row tiles 4G..4G+3)
            if G > 0:
                for ii in range(4):
                    x_unit(G, ii)
            em_qk(wqtv, qt, G, 0, qfv if SC_FP8 else None)
            em_qk(wktv, kt, G, 0, kfv if SC_FP8 else None)
            for i in range(4 * G, 4 * G + 4):
                em_v(i, 0)

        # ---- Main loop: per head pair, attention for its two heads.
        # Pair t+1's projections (and for the last pair, the O-projection)
        # interleave into the ACT-bound attention stream to keep PE busy.
        NY = DT // 2 if QKV_FP8 else DT  # yields per proj chain
        for t in range(NPAIR):
            # next-pair proj chains in x-chunk-arrival order (c-major): during
            # pair 0 the pulls then only ever touch xt chunks <= the current
            # G's (whose x_unit is already emitted), so no pulled instruction
            # can wait on a transpose emitted later (deadlock-safe).
            gens = []
            if t + 1 < NPAIR:
                for c in range(4):
                    gens.append(
                        gen_qk(wqtv, qt, c, t + 1, qfv if SC_FP8 else None)
                    )
                    gens.append(
                        gen_qk(wktv, kt, c, t + 1, kfv if SC_FP8 else None)
                    )
                    for i in range(4 * c, 4 * c + 4):
                        gens.append(gen_v(i, t + 1))
            gens.reverse()

            def pull(n):
                for _ in range(n):
                    while gens:
                        try:
                            next(gens[-1])
                            break
                        except StopIteration:
                            gens.pop()
                    if not gens:
                        return

            n_yields = (8 + ST) * NY  # total yields for next pair's chains
            # pair 0: exactly one c-block (48 yields) per G = 24 per half,
            # matching the safe prefix after prelude(G).
            per_half = 24 if t == 0 else (n_yields + 7) // 8 + 1

            g_order = (0, 1, 2, 3) if t == 0 else (3, 2, 1, 0)
            for G in g_order:
                if t == 0:
                    prelude(G)
                njs = 4 * G + 4  # sk-tiles 0..4G+3
                M = njs // 2  # score-tile pairs
                tpq = psM.tile([P, 512], F16, tag="tpq", bufs=1)
                for half in range(2):
                    if not (t == 0 and G == 0 and half == 0):
                        # (the very first half of pair 0 skips its pull so the
                        # first score->exp chain isn't queued behind 24 proj
                        # matmuls; the deferred budget drains at pair end)
                        pull(per_half)
                    h = 2 * t + half
                    pb = 64 * half
                    po = psM.tile([P, 4 * 65], F32, tag="po", bufs=1)
                    sts = [None] * M

                    def emit_st2(m):
                        # paired S^T: two k-tiles (j=2m, 2m+1) into one 2-bank
                        # PSUM tile; the odd tile is widened to the even
                        # tile's o so one Exp can cover both. With EXPP=0,
                        # per-j tiles/Exps (each k-tile at its own o).
                        o = max(0, (2 * m - 4 * G) * P)
                        st = psM.tile([P, 1024], F32, tag="st2")
                        for sub in range(2):
                            j = 2 * m + sub
                            # each sub-tile's matmul computes only its true
                            # causal range; the paired Exp still reads the
                            # widened [o:512) AP, so on the odd sub it exps
                            # stale PSUM in [o:oj) — harmless: PV skips i<j,
                            # and stale score-range data can't overflow f16
                            # under exp(x/8).
                            oj = max(0, (j - 4 * G) * P)
                            if SC_FP8:
                                fb = 64 * half
                                nc.tensor.matmul(
                                    st[:, sub * 512 + oj : sub * 512 + 512],
                                    lhsT=kfv[
                                        fb : fb + 32,
                                        t,
                                        :,
                                        j * P : (j + 1) * P,
                                    ],
                                    rhs=qfv[
                                        fb : fb + 32,
                                        t,
                                        :,
                                        G * 512 + oj : (G + 1) * 512,
                                    ],
                                    start=True,
                                    stop=True,
                                    perf_mode=DR,
                                )
                            else:
                                nc.tensor.matmul(
                                    st[:, sub * 512 + oj : sub * 512 + 512],
                                    lhsT=kt[
                                        pb : pb + 64,
                                        t * S + j * P : t * S + (j + 1) * P,
                                    ],
                                    rhs=qt[
                                        pb : pb + 64,
                                        t * S + G * 512 + oj : t * S + (G + 1) * 512,
                                    ],
                                    start=True,
                                    stop=True,
                                )
                        sts[m] = (st, o)

                    emit_st2(0)
                    for m in range(M):
                        if m + 1 < M:
                            emit_st2(m + 1)  # pipeline: next S^T before this PV
                        st, o = sts[m]
                        sts[m] = None
                        pts2 = work_tile(persist, "pts2", [P, 1024], PD, 6)
                        ptsv = pts2.rearrange("p (s c) -> p s c", s=2)
                        stv = st.rearrange("p (s c) -> p s c", s=2)
                        if EXPP:
                            nc.scalar.activation(
                                ptsv[:, :, o:512], stv[:, :, o:512], AF.Exp,
                                scale=SCALE,
                            )
                        else:
                            for sub in range(2):
                                j = 2 * m + sub
                                oj = max(0, (j - 4 * G) * P)
                                nc.scalar.activation(
                                    ptsv[:, sub, oj:512],
                                    stv[:, sub, oj:512],
                                    AF.Exp,
                                    scale=SCALE,
                                )
                        for sub in range(2):
                            j = 2 * m + sub
                            if j >= 4 * G:
                                # diagonal tile: keep q >= k (col >= partition)
                                oj = (j - 4 * G) * P
                                nc.gpsimd.affine_select(
                                    out=ptsv[:, sub, oj : oj + P],
                                    in_=ptsv[:, sub, oj : oj + P],
                                    compare_op=mybir.AluOpType.is_ge,
                                    fill=0.0,
                                    base=0,
                                    pattern=[[1, P]],
                                    channel_multiplier=-1,
                                )
                        for ii in range(4):
                            i = 4 * G + ii
                            # PSUM start=True clears has_written for the WHOLE
                            # bank, so only the first matmul of the tile sets
                            # it; later regions first-write onto cleared
                            # has_written (= overwrite).
                            if PV_FP8:
                                if 2 * m + 1 <= i:
                                    nc.tensor.matmul(
                                        po[:, ii * 65 : (ii + 1) * 65],
                                        lhsT=ptsv[:, :, ii * P : (ii + 1) * P],
                                        rhs=v4v[:, 2 * m : 2 * m + 2, h, :],
                                        start=(m == 0 and ii == 0),
                                        stop=(2 * m + 1 == i),
                                        perf_mode=DR,
                                        skip_group_check=True,
                                    )
                                elif 2 * m == i:
                                    nc.tensor.matmul(
                                        po[:, ii * 65 : (ii + 1) * 65],
                                        lhsT=ptsv[:, 0, ii * P : (ii + 1) * P],
                                        rhs=v4v[:, 2 * m, h, :],
                                        start=(m == 0 and ii == 0),
                                        stop=True,
                                        skip_group_check=True,
                                    )
                            else:
                                for sub in range(2):
                                    j = 2 * m + sub
                                    if i < j:
                                        continue
                                    nc.tensor.matmul(
                                        po[:, ii * 65 : (ii + 1) * 65],
                                        lhsT=ptsv[:, sub, ii * P : (ii + 1) * P],
                                        rhs=v4v[:, j, h, :],
                                        start=(m == 0 and sub == 0 and ii == 0),
                                        stop=(j == i),
                                        skip_group_check=True,
                                    )
                    # normalize: aon = po[:, i, 0:64] * (1/po[:, i, 64])
                    pov = po.rearrange("p (i e) -> p i e", i=4)
                    rec = work_tile(persist, "rec", [P, 4], F32, 3)
                    nc.vector.reciprocal(rec, pov[:, :, 64])
                    aon = work_tile(persist, "aon", [P, 256], F16, 3)
                    nc.vector.tensor_tensor(
                        out=aon.rearrange("p (i e) -> p i e", i=4),
                        in0=pov[:, :, 0:64],
                        in1=rec.to_broadcast((P, 4, 64)),
                        op=mybir.AluOpType.mult,
                    )
                    # transpose AO [sq,64] -> [64,sq] into partition half pb
                    for ii in range(4):
                        nc.tensor.transpose(
                            tpq[pb : pb + 64, ii * P : (ii + 1) * P],
                            aon[:, ii * 64 : (ii + 1) * 64],
                            ident,
                        )
                nc.vector.tensor_copy(
                    aot[:, t * S + G * 512 : t * S + (G + 1) * 512], tpq
                )
                if t == NPAIR - 1:
                    # queue this sq group's O-projection as pull-gens: its
                    # matmuls interleave into the NEXT G group's exp-gated
                    # attention stream (pair 3 has no next-pair projections to
                    # pull, and emitting the O-proj as a solid block leaves
                    # the attention windows at ~50% PE duty — which trips the
                    # HAM activity throttle to K=4/8 for ~24us)
                    for ii in range(4):
                        gens.append(gen_oproj(4 * G + ii))
            pull(10 ** 6)  # drain remaining proj / O-proj work
        stage.release()
        if dbg is not None:
            nc.gpsimd.dma_start(out=dbg["qt"][:, :], in_=qt)
            nc.gpsimd.dma_start(out=dbg["kt"][:, :], in_=kt)
            nc.gpsimd.dma_start(out=dbg["aot"][:, :], in_=aot)
            nc.gpsimd.dma_start(out=dbg["v4"][:, :], in_=v4)
            if SC_FP8:
                nc.gpsimd.dma_start(out=dbg["qf"][:, :], in_=qf)
                nc.gpsimd.dma_start(out=dbg["kf"][:, :], in_=kf)


def build_nc(debug=False):
    nc = bass.Bass()
    # single input blob (fewer per-call PJRT args = less host dispatch cost):
    # rows [0,S) = x, then 512 rows each of W_Q, W_K, W_V, W_O
    blob = nc.dram_tensor("blob", [S + 4 * NH * H, D], F32, kind="ExternalInput")
    x = blob[0:S, :]
    wq_d = blob[S : S + 512, :]
    wk_d = blob[S + 512 : S + 1024, :]
    wv_d = blob[S + 1024 : S + 1536, :]
    wo_d = blob[S + 1536 : S + 2048, :]
    out_d = nc.dram_tensor("out", [S, D], F32, kind="ExternalOutput")
    dbg = None
    if debug:
        QKD = F8 if SC_FP8 else F16
        dbg = {
            "qt": nc.dram_tensor("dbg_qt", [P, NPAIR * S], QKD, kind="ExternalOutput"),
            "kt": nc.dram_tensor("dbg_kt", [P, NPAIR * S], QKD, kind="ExternalOutput"),
            "aot": nc.dram_tensor("dbg_aot", [P, NPAIR * S], F16, kind="ExternalOutput"),
            "v4": nc.dram_tensor("dbg_v4", [P, ST * NH * 65], PD, kind="ExternalOutput"),
        }
        if SC_FP8:
            dbg["qf"] = nc.dram_tensor("dbg_qf", [P, NPAIR * 2 * S], F8, kind="ExternalOutput")
            dbg["kf"] = nc.dram_tensor("dbg_kf", [P, NPAIR * 2 * S], F8, kind="ExternalOutput")
    with tile.TileContext(nc) as tc:
        emit(nc, tc, x, wq_d, wk_d, wv_d, wo_d, out_d, dbg=dbg)
    split_excess_waits(nc, compute_max_waits=COMPUTE_MAX_WAITS)
    return nc


_cache = {}


def _get_runner():
    """Persistent jitted 8-core runner (mirrors bass2jax.run_bass_via_pjrt's
    multi-core path, but reusable across calls so we can time executions)."""
    if "runner" in _cache:
        return _cache["runner"]
    import jax
    from jax.experimental.shard_map import shard_map
    from jax.sharding import Mesh, PartitionSpec
    from concourse import bass2jax

    bass2jax.install_neuronx_cc_hook()
    if "nc" not in _cache:
        _cache["nc"] = build_nc()
    nc = _cache["nc"]

    partition_name = nc.partition_id_tensor.name if nc.partition_id_tensor else None
    in_names, out_names, out_avals = [], [], []
    for alloc in nc.m.functions[0].allocations:
        if not isinstance(alloc, mybir.MemoryLocationSet):
            continue
        name = alloc.memorylocations[0].name
        if alloc.kind == "ExternalInput":
            if name != partition_name:
                in_names.append(name)
        elif alloc.kind == "ExternalOutput":
            out_names.append(name)
            out_avals.append(
                jax.core.ShapedArray(tuple(alloc.tensor_shape), mybir.dt.np(alloc.dtype))
            )
    n_params, n_outs = len(in_names), len(out_names)
    all_names = list(in_names) + list(out_names)
    if partition_name is not None:
        all_names.append(partition_name)
    all_names = tuple(all_names)

    def _body(*args):
        operands = list(args)
        if partition_name is not None:
            operands.append(bass2jax.partition_id_tensor())
        outs = bass2jax._bass_exec_p.bind(
            *operands,
            out_avals=tuple(out_avals),
            in_names=all_names,
            out_names=tuple(out_names),
            lowering_input_output_aliases=(),
            sim_require_finite=True,
            sim_require_nnan=True,
            nc=nc,
        )
        return tuple(outs)

    devices = jax.devices()[:8]
    mesh = Mesh(np.asarray(devices), ("core",))
    in_specs = (PartitionSpec("core"),) * (n_params + n_outs)
    out_specs = (PartitionSpec("core"),) * n_outs
    donate = tuple(range(n_params, n_params + n_outs))
    sharded = jax.jit(
        shard_map(_body, mesh=mesh, in_specs=in_specs, out_specs=out_specs, check_rep=False),
        donate_argnums=donate,
        keep_unused=True,
    )
    _cache["runner"] = (sharded, in_names, out_names, out_avals, mesh)
    return _cache["runner"]


def run_on_cores(in_maps):
    """Run the kernel on 8 cores; returns list of per-core output dicts."""
    sharded, in_names, out_names, out_avals, mesh = _get_runner()
    concat_in = [
        np.concatenate([np.asarray(in_maps[c][name]) for c in range(8)], axis=0)
        for name in in_names
    ]
    concat_zeros = [
        np.zeros((8 * a.shape[0], *a.shape[1:]), a.dtype) for a in out_avals
    ]
    out_arrs = sharded(*concat_in, *concat_zeros)
    return [
        {
            name: np.asarray(out_arrs[i]).reshape(8, *out_avals[i].shape)[c]
            for i, name in enumerate(out_names)
        }
        for c in range(8)
    ]


def make_in_maps(residual, W_Q, W_K, W_V, W_O):
    in_maps = []
    for core in range(8):
        b, g = core // 2, core % 2
        sl = slice(8 * g, 8 * (g + 1))
        blob = np.concatenate(
            [
                np.asarray(residual[b], dtype=np.float32).reshape(S, D),
                np.asarray(W_Q[sl], dtype=np.float32).reshape(NH * H, D),
                np.asarray(W_K[sl], dtype=np.float32).reshape(NH * H, D),
                np.asarray(W_V[sl], dtype=np.float32).reshape(NH * H, D),
                np.asarray(W_O[sl], dtype=np.float32).reshape(NH * H, D),
            ],
            axis=0,
        )
        in_maps.append({"blob": np.ascontiguousarray(blob)})
    return in_maps


def kernel(residual, W_Q, W_K, W_V, W_O):
    residual = np.asarray(residual)
    in_maps = make_in_maps(residual, W_Q, W_K, W_V, W_O)
    results = run_on_cores(in_maps)
    B = residual.shape[0]
    out = np.zeros((B, S, D), np.float32)
    for core in range(8):
        b = core // 2
        out[b] += results[core]["out"]
    return out


if __name__ == "__main__":
    rng = np.random.default_rng(0)
    residual = rng.standard_normal((4, S, D)).astype(np.float32)
    W = [0.02 * rng.standard_normal((16, H, D)).astype(np.float32) for _ in range(4)]
    out = kernel(residual, *W)
    print("kernel ran, out shape", out.shape, "finite:", np.isfinite(out).all())



# revision 19
# speedup vs baseline: 1.1046x; 1.1046x over previous
"""Trainium2 Bass kernel for causal multi-head attention (B=4,S=2048,D=1024,N=16,H=64).

Sharding: 8 cores = (batch b in 0..3) x (head-group g in 0..1). Each core gets
residual[b] and 8 heads' worth of W_Q/K/V/O, computes the partial output
sum_{n in group} attn_n @ W_O[n]  ->  [2048,1024]; host adds the two
head-group partials per batch. No collectives needed.

v3 structure (per core, all-f16 matmuls):
  - Inputs cast fp32->fp16 during the DMA itself (SWDGE gpsimd loads), ALL
    kicked back-to-back at program start so the ring streams the 16MB of
    reads gap-free. All transposes (W^T and X^T) are f16 PE transposes +
    DVE evacs: XBAR DMA-transpose consumers made the tile scheduler
    (which models DMA as one serial resource) interleave + semaphore-
    serialize the loads, doubling phase-1 time.
  - A ~30-matmul PE warm-up spin at t=0 keeps the HAM clock-gate at 8/8
    (otherwise the load-gated prologue runs the PE at 1.2GHz).
  - Pair 0 streams in ASCENDING G order: per-G prelude transposes x group
    G and runs the projections it unblocks, so attention starts as soon as
    the first x chunk lands instead of after the full load.
  - Scores S^T = K^T.T @ Q^T, PAIRED: two k-tiles' scores land in one
    2-bank PSUM tile and ONE Exp instruction covers both ([128, 2, 512-o]),
    halving ACT per-instruction overhead (the dominant HW win, ~100us).
    Each sub-tile's matmul computes only its true causal range; the Exp
    reads the widened AP (stale PSUM on the odd tile is never consumed).
  - PV with a ones column per head (softmax denominator on PE for free).
  - Normalize on DVE, transpose AO on PE, O-projection f16 pulled into
    pair 3's exp-gated attention gaps as generators (a solid O-proj block
    left the attention windows at ~50% PE duty, tripping the HAM activity
    throttle to half clock for ~24us).
  - Inputs arrive as ONE dram blob (x + 4 W's): fewer per-call PJRT args
    cut the host dispatch cost of each execution ~460us -> ~300us.
  fp8 variants (DoubleRow proj/PV/scores) are implemented behind env flags
  but default OFF: plain fp8 evac+matmul produces NaN on this HW stack
  despite passing CoreSim and isolated-primitive HW probes; fp8 proj/PV
  also exceed the 2e-2 error budget (4.7e-2 / 2.5e-2).
"""

import sys

sys.path.insert(0, "/opt/trn_rl_repo")

import numpy as np
import concourse.bass as bass
import concourse.mybir as mybir
import concourse.tile as tile
from concourse.bass_utils import run_bass_kernel_spmd
from concourse.masks import make_identity

F32 = mybir.dt.float32
F32R = mybir.dt.float32r
F16 = mybir.dt.float16
F8 = mybir.dt.float8e4
AF = mybir.ActivationFunctionType
DR = mybir.MatmulPerfMode.DoubleRow

S = 2048
D = 1024
NH = 8  # heads per core
H = 64
P = 128
ST = S // P  # 16
DT = D // P  # 8
NPAIR = NH // 2  # 4
SCALE = 1.0 / 8.0  # 1/sqrt(H)
COMPUTE_MAX_WAITS = 1

import os

QKV_FP8 = os.environ.get("K_QKV_FP8", "0") == "1"  # fp8 DoubleRow projections
PV_FP8 = os.environ.get("K_PV_FP8", "0") == "1"  # fp8 P^T + fp8 V, DoubleRow PV
PH1 = os.environ.get("K_PH1", "xbar")  # "xbar" | "pe" phase-1 scheme
EXPP = os.environ.get("K_EXPP", "1") == "1"  # paired 2-bank Exp
SC_MODE = os.environ.get("K_SC", "f16")  # f16 | plain8 | dr
SC_FP8 = SC_MODE == "dr"  # fp8 DoubleRow scores (fold + DR)
SC_P8 = SC_MODE in ("plain8", "dr")  # fp8 qt/kt evac

XW_DT = F8 if QKV_FP8 else F16
PD = F8 if PV_FP8 else F16  # pattern dtype

CTRL_INSTS = ("InstDrain", "InstNop", "InstEventSemaphoreOp", "InstSemaphoreOp")


def split_excess_waits(nc, max_waits=1, compute_max_waits=1):
    """This walrus build rejects >1 sync wait on CTRL-class instructions
    (Drain/NoOp). Move excess waits onto same-engine NOPs inserted
    immediately before. Compute instructions may keep compute_max_waits."""
    n_split = 0
    for bb in nc.main_func.blocks:
        insts = list(bb.instructions)
        out = []
        for ins in insts:
            si = ins.sync_info
            lim = max_waits if type(ins).__name__ in CTRL_INSTS else compute_max_waits
            if si is not None and si.on_wait and len(si.on_wait) > lim:
                waits = list(si.on_wait)
                while len(waits) > lim:
                    chunk, waits = waits[:1], waits[1:]
                    nop = nc.engines[ins.engine].nop(nofuse=True).ins
                    for b2 in nc.main_func.blocks:
                        if nop in b2.instructions:
                            b2.instructions.remove(nop)
                            break
                    if nop.sync_info is None:
                        nop.sync_info = mybir.SyncInfo(on_wait=[], on_update=[])
                    nop.sync_info.on_wait = chunk
                    out.append(nop)
                    n_split += 1
                si.on_wait = waits
            out.append(ins)
        bb.instructions[:] = out
    return n_split


def work_tile(pool, tag, shape, dtype, bufs):
    return pool.tile(shape, dtype, tag=tag, bufs=bufs, name=tag)


def emit(nc, tc, x, wq_d, wk_d, wv_d, wo_d, out_d, dbg=None):
    with (
        tc.tile_pool(name="const", bufs=1) as constp,
        tc.tile_pool(name="persist", bufs=1) as persist,
        tc.tile_pool(name="psM", bufs=2, space="PSUM") as psM,
    ):
        ident = constp.tile([P, P], F16)

        xt = persist.tile([P, DT * S], XW_DT)
        wqt = persist.tile([P, DT * 512], XW_DT)
        wkt = persist.tile([P, DT * 512], XW_DT)
        wvt = persist.tile([P, DT * 512], XW_DT)
        QK_DT = F8 if SC_P8 else F16
        qt = persist.tile([P, NPAIR * S], QK_DT)
        kt = persist.tile([P, NPAIR * S], QK_DT)
        qfv = kfv = None
        if SC_FP8:
            # folded layouts for DoubleRow scores: [32-part head strips x
            # 2 k-subtiles]; head-half hh at partition base 64*hh, pair t in
            # column block t. h-dim index = 32*i + p within each strip.
            qf = persist.tile([P, NPAIR * 2 * S], F8)
            kf = persist.tile([P, NPAIR * 2 * S], F8)
            qfv = qf.rearrange("p (b i s) -> p b i s", b=NPAIR, i=2)
            kfv = kf.rearrange("p (b i s) -> p b i s", b=NPAIR, i=2)

        v4 = persist.tile([P, ST * NH * 65], PD)
        aot = persist.tile([P, NPAIR * S], F16)
        wo_sb = persist.tile([P, NPAIR * D], F16)

        v4v = v4.rearrange("p (i h e) -> p i h e", i=ST, h=NH)
        xtv = xt.rearrange("p (k s) -> p k s", k=DT)
        wqtv = wqt.rearrange("p (k h) -> p k h", k=DT)
        wktv = wkt.rearrange("p (k h) -> p k h", k=DT)
        wvtv = wvt.rearrange("p (k h) -> p k h", k=DT)

        # projection-chain generators: yield after each matmul so the pair
        # loop can interleave single proj matmuls into ACT-bound gaps
        def gen_qk(wtv_src, dst, c, t, fold_dstv=None):
            pq = psM.tile([P, 512], F32, tag="proj")
            if QKV_FP8:
                for k2 in range(DT // 2):
                    nc.tensor.matmul(
                        pq,
                        lhsT=wtv_src[:, 2 * k2 : 2 * k2 + 2, t * P : (t + 1) * P],
                        rhs=xtv[:, 2 * k2 : 2 * k2 + 2, c * 512 : (c + 1) * 512],
                        start=(k2 == 0),
                        stop=(k2 == DT // 2 - 1),
                        perf_mode=DR,
                    )
                    yield
            else:
                for k in range(DT):
                    nc.tensor.matmul(
                        pq,
                        lhsT=wtv_src[:, k, t * P : (t + 1) * P],
                        rhs=xtv[:, k, c * 512 : (c + 1) * 512],
                        start=(k == 0),
                        stop=(k == DT - 1),
                    )
                    yield
            nc.vector.tensor_copy(
                dst[:, t * S + c * 512 : t * S + (c + 1) * 512], pq
            )
            if fold_dstv is not None:
                # partition-fold this chunk [64 h, 512] -> [32, 2, 512]
                for hh in range(2):
                    for i2 in range(2):
                        nc.sync.dma_start(
                            out=fold_dstv[
                                64 * hh : 64 * hh + 32,
                                t,
                                i2,
                                c * 512 : (c + 1) * 512,
                            ],
                            in_=dst[
                                64 * hh + 32 * i2 : 64 * hh + 32 * i2 + 32,
                                t * S + c * 512 : t * S + (c + 1) * 512,
                            ],
                        )

        def gen_v(i, t):
            pv = psM.tile([P, 512], F32, tag="proj")
            if QKV_FP8:
                for k2 in range(DT // 2):
                    nc.tensor.matmul(
                        pv[:, 0:P],
                        lhsT=xtv[:, 2 * k2 : 2 * k2 + 2, i * P : (i + 1) * P],
                        rhs=wvtv[:, 2 * k2 : 2 * k2 + 2, t * P : (t + 1) * P],
                        start=(k2 == 0),
                        stop=(k2 == DT // 2 - 1),
                        perf_mode=DR,
                    )
                    yield
            else:
                for k in range(DT):
                    nc.tensor.matmul(
                        pv[:, 0:P],
                        lhsT=xtv[:, k, i * P : (i + 1) * P],
                        rhs=wvtv[:, k, t * P : (t + 1) * P],
                        start=(k == 0),
                        stop=(k == DT - 1),
                    )
                    yield
            nc.vector.tensor_copy(
                v4v[:, i, 2 * t : 2 * t + 2, 0:64],
                pv[:, 0:P].rearrange("p (h e) -> p h e", h=2),
            )

        def gen_oproj(i):
            # O-projection for row tile i: out[i] = sum_t aot[t,i]^T @ wo[t];
            # both 512-col chunks batched into one 512KB store
            osb = work_tile(persist, "osb", [P, D], F32, 2)
            for c in range(2):
                oo = psM.tile([P, 512], F32, tag="proj")
                for tp in range(NPAIR):
                    nc.tensor.matmul(
                        oo,
                        lhsT=aot[:, tp * S + i * P : tp * S + (i + 1) * P],
                        rhs=wo_sb[:, tp * D + c * 512 : tp * D + (c + 1) * 512],
                        start=(tp == 0),
                        stop=(tp == NPAIR - 1),
                    )
                    yield
                nc.vector.tensor_copy(osb[:, c * 512 : (c + 1) * 512], oo)
            nc.sync.dma_start(out=out_d[i * P : (i + 1) * P, :], in_=osb)

        def em_qk(wtv_src, dst, c, t, fold_dstv=None):
            for _ in gen_qk(wtv_src, dst, c, t, fold_dstv):
                pass

        def em_v(i, t):
            for _ in gen_v(i, t):
                pass

        # ---- Phase 1: SWDGE cast-loads (fp32->fp16 in DMA), all kicked
        # back-to-back at program start. ALL transposes (W^T and X^T) are f16
        # PE transposes: with only PE/DVE consumers the tile scheduler leaves
        # the SWDGE kicks wait-free, so the ring streams the 16MB of input
        # reads gap-free (XBAR transposes made the scheduler interleave +
        # semaphore-serialize the loads, stretching phase 1 2x). A PE warm-up
        # spin covers the first ~8us so the HAM clock-gate is at 8/8 before
        # real compute starts (otherwise the whole prologue runs at 1.2GHz).
        stage = tc.alloc_tile_pool(name="stage", bufs=1)
        # PE warm-up spin: ~30 512-col matmuls on a zeroed f16 tile; the
        # products land in a scratch PSUM bank nobody reads
        spin_src = constp.tile([P, 512], F16)
        nc.vector.memset(spin_src, 0.0)
        spin_ps = psM.tile([P, 1024], F32, tag="st2")
        for _ in range(30):
            nc.tensor.matmul(
                spin_ps[:, 0:512],
                lhsT=spin_src[:, 0:P],
                rhs=spin_src,
                start=True,
                stop=True,
            )
        # loads in consumption order: x group 0 first (it gates the whole
        # pair-0 stream), then W_Q/K/V, then x groups 1-3, then W_O.
        xf_by_g = {}

        def xload(ig):
            xf = stage.tile([P, 4 * D], F16, tag="xf", bufs=4)
            nc.gpsimd.dma_start(
                out=xf,
                in_=x[ig * 4 * P : (ig + 1) * 4 * P, :].rearrange(
                    "(a p) d -> p a d", p=P
                ),
            )
            xf_by_g[ig] = xf.rearrange("p (a d) -> p a d", a=4)

        xload(0)
        wq16 = stage.tile([P, 4 * D], F16, tag="w16", bufs=3)
        nc.gpsimd.dma_start(out=wq16, in_=wq_d.rearrange("(a p) d -> p a d", p=P))
        wk16 = stage.tile([P, 4 * D], F16, tag="w16", bufs=3)
        nc.gpsimd.dma_start(out=wk16, in_=wk_d.rearrange("(a p) d -> p a d", p=P))
        wv16 = stage.tile([P, 4 * D], F16, tag="w16", bufs=3)
        nc.gpsimd.dma_start(out=wv16, in_=wv_d.rearrange("(a p) d -> p a d", p=P))
        for ig in range(1, 4):
            xload(ig)
        # W_O: cast-load directly into its final SBUF layout
        nc.gpsimd.dma_start(
            out=wo_sb.rearrange("p (a d) -> p a d", a=NPAIR),
            in_=wo_d.rearrange("(a p) d -> p a d", p=P),
        )

        # gpsimd-engine setup, after all the DMA kicks
        make_identity(nc, ident)
        nc.gpsimd.memset(v4v[:, :, :, 64], 1.0)

        def x_unit(ig, ii):
            # f16 PE transpose of [128 s, 1024 d] + DVE evac
            st8 = 4 * ig + ii
            pt = psM.tile([P, 1024], F16, tag="st2")
            for k in range(DT):
                nc.tensor.transpose(
                    pt[:, k * P : (k + 1) * P],
                    xf_by_g[ig][:, ii, k * P : (k + 1) * P],
                    ident,
                )
            nc.vector.tensor_copy(
                xtv[:, 0:DT, st8 * P : (st8 + 1) * P],
                pt.rearrange("p (k c) -> p k c", k=DT),
            )

        def w_unit(w16v, wtv_dst, j):
            # f16 PE transpose [128 h, 1024 d] -> [128 d, 8k, 128 h]
            pt = psM.tile([P, 1024], F16, tag="st2")
            for k in range(DT):
                nc.tensor.transpose(
                    pt[:, k * P : (k + 1) * P],
                    w16v[:, j, k * P : (k + 1) * P],
                    ident,
                )
            nc.vector.tensor_copy(
                wtv_dst[:, 0:DT, j * P : (j + 1) * P],
                pt.rearrange("p (k c) -> p k c", k=DT),
            )

        wq16v = wq16.rearrange("p (a d) -> p a d", a=4)
        wk16v = wk16.rearrange("p (a d) -> p a d", a=4)
        wv16v = wv16.rearrange("p (a d) -> p a d", a=4)

        # PE order = data-arrival order: x0 transposes, then W^T (j=0 blocks
        # first: they alone gate pair-0's projections)
        for ii in range(4):
            x_unit(0, ii)
        for j in range(4):
            w_unit(wq16v, wqtv, j)
        for j in range(4):
            w_unit(wk16v, wktv, j)
        for j in range(4):
            w_unit(wv16v, wvtv, j)

        def prelude(G):
            # pair-0 per-G streaming prelude: transpose x group G, then the
            # projections it unblocks (Q/K chunk c=G, V row tiles 4G..4G+3)
            if G > 0:
                for ii in range(4):
                    x_unit(G, ii)
            em_qk(wqtv, qt, G, 0, qfv if SC_FP8 else None)
            em_qk(wktv, kt, G, 0, kfv if SC_FP8 else None)
            for i in range(4 * G, 4 * G + 4):
                em_v(i, 0)

        # ---- Main loop: per head pair, attention for its two heads.
        # Pair t+1's projections (and for the last pair, the O-projection)
        # interleave into the ACT-bound attention stream to keep PE busy.
        NY = DT // 2 if QKV_FP8 else DT  # yields per proj chain
        for t in range(NPAIR):
            # next-pair proj chains in x-chunk-arrival order (c-major): during
            # pair 0 the pulls then only ever touch xt chunks <= the current
            # G's (whose x_unit is already emitted), so no pulled instruction
            # can wait on a transpose emitted later (deadlock-safe).
            gens = []
            if t + 1 < NPAIR:
                for c in range(4):
                    gens.append(
                        gen_qk(wqtv, qt, c, t + 1, qfv if SC_FP8 else None)
                    )
                    gens.append(
                        gen_qk(wktv, kt, c, t + 1, kfv if SC_FP8 else None)
                    )
                    for i in range(4 * c, 4 * c + 4):
                        gens.append(gen_v(i, t + 1))
            gens.reverse()

            def pull(n):
                for _ in range(n):
                    while gens:
                        try:
                            next(gens[-1])
                            break
                        except StopIteration:
                            gens.pop()
                    if not gens:
                        return

            n_yields = (8 + ST) * NY  # total yields for next pair's chains
            # pair 0: exactly one c-block (48 yields) per G = 24 per half,
            # matching the safe prefix after prelude(G).
            per_half = 24 if t == 0 else (n_yields + 7) // 8 + 1

            # pair 0 ascends so compute streams with x-chunk arrival; pair 3
            # ascends so each G group's O-proj gens are queued as PE filler
            # for the NEXT (bigger) group's exp-gated attention windows — the
            # big G=3 group then runs with ~20us of filler instead of none
            # (descending left G=3 filler-less at ~50% PE duty, tripping the
            # HAM activity throttle to K=4 for ~24us at t~252us).
            g_order = (0, 1, 2, 3) if t in (0, NPAIR - 1) else (3, 2, 1, 0)
            for G in g_order:
                if t == 0:
                    prelude(G)
                njs = 4 * G + 4  # sk-tiles 0..4G+3
                M = njs // 2  # score-tile pairs
                tpq = psM.tile([P, 512], F16, tag="tpq", bufs=1)
                for half in range(2):
                    if not (t == 0 and G == 0 and half == 0):
                        # (the very first half of pair 0 skips its pull so the
                        # first score->exp chain isn't queued behind 24 proj
                        # matmuls; the deferred budget drains at pair end)
                        pull(per_half)
                    h = 2 * t + half
                    pb = 64 * half
                    po = psM.tile([P, 4 * 65], F32, tag="po", bufs=1)
                    sts = [None] * M

                    def emit_st2(m):
                        # paired S^T: two k-tiles (j=2m, 2m+1) into one 2-bank
                        # PSUM tile; the odd tile is widened to the even
                        # tile's o so one Exp can cover both. With EXPP=0,
                        # per-j tiles/Exps (each k-tile at its own o).
                        o = max(0, (2 * m - 4 * G) * P)
                        st = psM.tile([P, 1024], F32, tag="st2")
                        for sub in range(2):
                            j = 2 * m + sub
                            # each sub-tile's matmul computes only its true
                            # causal range; the paired Exp still reads the
                            # widened [o:512) AP, so on the odd sub it exps
                            # stale PSUM in [o:oj) — harmless: PV skips i<j,
                            # and stale score-range data can't overflow f16
                            # under exp(x/8).
                            oj = max(0, (j - 4 * G) * P)
                            if SC_FP8:
                                fb = 64 * half
                                nc.tensor.matmul(
                                    st[:, sub * 512 + oj : sub * 512 + 512],
                                    lhsT=kfv[
                                        fb : fb + 32,
                                        t,
                                        :,
                                        j * P : (j + 1) * P,
                                    ],
                                    rhs=qfv[
                                        fb : fb + 32,
                                        t,
                                        :,
                                        G * 512 + oj : (G + 1) * 512,
                                    ],
                                    start=True,
                                    stop=True,
                                    perf_mode=DR,
                                )
                            else:
                                nc.tensor.matmul(
                                    st[:, sub * 512 + oj : sub * 512 + 512],
                                    lhsT=kt[
                                        pb : pb + 64,
                                        t * S + j * P : t * S + (j + 1) * P,
                                    ],
                                    rhs=qt[
                                        pb : pb + 64,
                                        t * S + G * 512 + oj : t * S + (G + 1) * 512,
                                    ],
                                    start=True,
                                    stop=True,
                                )
                        sts[m] = (st, o)

                    emit_st2(0)
                    for m in range(M):
                        if m + 1 < M:
                            emit_st2(m + 1)  # pipeline: next S^T before this PV
                        st, o = sts[m]
                        sts[m] = None
                        pts2 = work_tile(persist, "pts2", [P, 1024], PD, 6)
                        ptsv = pts2.rearrange("p (s c) -> p s c", s=2)
                        stv = st.rearrange("p (s c) -> p s c", s=2)
                        if EXPP:
                            nc.scalar.activation(
                                ptsv[:, :, o:512], stv[:, :, o:512], AF.Exp,
                                scale=SCALE,
                            )
                        else:
                            for sub in range(2):
                                j = 2 * m + sub
                                oj = max(0, (j - 4 * G) * P)
                                nc.scalar.activation(
                                    ptsv[:, sub, oj:512],
                                    stv[:, sub, oj:512],
                                    AF.Exp,
                                    scale=SCALE,
                                )
                        for sub in range(2):
                            j = 2 * m + sub
                            if j >= 4 * G:
                                # diagonal tile: keep q >= k (col >= partition)
                                oj = (j - 4 * G) * P
                                nc.gpsimd.affine_select(
                                    out=ptsv[:, sub, oj : oj + P],
                                    in_=ptsv[:, sub, oj : oj + P],
                                    compare_op=mybir.AluOpType.is_ge,
                                    fill=0.0,
                                    base=0,
                                    pattern=[[1, P]],
                                    channel_multiplier=-1,
                                )
                        for ii in range(4):
                            i = 4 * G + ii
                            # PSUM start=True clears has_written for the WHOLE
                            # bank, so only the first matmul of the tile sets
                            # it; later regions first-write onto cleared
                            # has_written (= overwrite).
                            if PV_FP8:
                                if 2 * m + 1 <= i:
                                    nc.tensor.matmul(
                                        po[:, ii * 65 : (ii + 1) * 65],
                                        lhsT=ptsv[:, :, ii * P : (ii + 1) * P],
                                        rhs=v4v[:, 2 * m : 2 * m + 2, h, :],
                                        start=(m == 0 and ii == 0),
                                        stop=(2 * m + 1 == i),
                                        perf_mode=DR,
                                        skip_group_check=True,
                                    )
                                elif 2 * m == i:
                                    nc.tensor.matmul(
                                        po[:, ii * 65 : (ii + 1) * 65],
                                        lhsT=ptsv[:, 0, ii * P : (ii + 1) * P],
                                        rhs=v4v[:, 2 * m, h, :],
                                        start=(m == 0 and ii == 0),
                                        stop=True,
                                        skip_group_check=True,
                                    )
                            else:
                                for sub in range(2):
                                    j = 2 * m + sub
                                    if i < j:
                                        continue
                                    nc.tensor.matmul(
                                        po[:, ii * 65 : (ii + 1) * 65],
                                        lhsT=ptsv[:, sub, ii * P : (ii + 1) * P],
                                        rhs=v4v[:, j, h, :],
                                        start=(m == 0 and sub == 0 and ii == 0),
                                        stop=(j == i),
                                        skip_group_check=True,
                                    )
                    # normalize: aon = po[:, i, 0:64] * (1/po[:, i, 64])
                    pov = po.rearrange("p (i e) -> p i e", i=4)
                    rec = work_tile(persist, "rec", [P, 4], F32, 3)
                    nc.vector.reciprocal(rec, pov[:, :, 64])
                    aon = work_tile(persist, "aon", [P, 256], F16, 3)
                    nc.vector.tensor_tensor(
                        out=aon.rearrange("p (i e) -> p i e", i=4),
                        in0=pov[:, :, 0:64],
                        in1=rec.to_broadcast((P, 4, 64)),
                        op=mybir.AluOpType.mult,
                    )
                    # transpose AO [sq,64] -> [64,sq] into partition half pb
                    for ii in range(4):
                        nc.tensor.transpose(
                            tpq[pb : pb + 64, ii * P : (ii + 1) * P],
                            aon[:, ii * 64 : (ii + 1) * 64],
                            ident,
                        )
                nc.vector.tensor_copy(
                    aot[:, t * S + G * 512 : t * S + (G + 1) * 512], tpq
                )
                if t == NPAIR - 1:
                    # queue this sq group's O-projection as pull-gens: its
                    # matmuls interleave into the NEXT G group's exp-gated
                    # attention stream (pair 3 has no next-pair projections to
                    # pull, and emitting the O-proj as a solid block leaves
                    # the attention windows at ~50% PE duty — which trips the
                    # HAM activity throttle to K=4/8 for ~24us)
                    for ii in range(4):
                        gens.append(gen_oproj(4 * G + ii))
            pull(10 ** 6)  # drain remaining proj / O-proj work
        stage.release()
        if dbg is not None:
            nc.gpsimd.dma_start(out=dbg["qt"][:, :], in_=qt)
            nc.gpsimd.dma_start(out=dbg["kt"][:, :], in_=kt)
            nc.gpsimd.dma_start(out=dbg["aot"][:, :], in_=aot)
            nc.gpsimd.dma_start(out=dbg["v4"][:, :], in_=v4)
            if SC_FP8:
                nc.gpsimd.dma_start(out=dbg["qf"][:, :], in_=qf)
                nc.gpsimd.dma_start(out=dbg["kf"][:, :], in_=kf)


def build_nc(debug=False):
    nc = bass.Bass()
    # single input blob (fewer per-call PJRT args = less host dispatch cost):
    # rows [0,S) = x, then 512 rows each of W_Q, W_K, W_V, W_O
    blob = nc.dram_tensor("blob", [S + 4 * NH * H, D], F32, kind="ExternalInput")
    x = blob[0:S, :]
    wq_d = blob[S : S + 512, :]
    wk_d = blob[S + 512 : S + 1024, :]
    wv_d = blob[S + 1024 : S + 1536, :]
    wo_d = blob[S + 1536 : S + 2048, :]
    out_d = nc.dram_tensor("out", [S, D], F32, kind="ExternalOutput")
    dbg = None
    if debug:
        QKD = F8 if SC_FP8 else F16
        dbg = {
            "qt": nc.dram_tensor("dbg_qt", [P, NPAIR * S], QKD, kind="ExternalOutput"),
            "kt": nc.dram_tensor("dbg_kt", [P, NPAIR * S], QKD, kind="ExternalOutput"),
            "aot": nc.dram_tensor("dbg_aot", [P, NPAIR * S], F16, kind="ExternalOutput"),
            "v4": nc.dram_tensor("dbg_v4", [P, ST * NH * 65], PD, kind="ExternalOutput"),
        }
        if SC_FP8:
            dbg["qf"] = nc.dram_tensor("dbg_qf", [P, NPAIR * 2 * S], F8, kind="ExternalOutput")
            dbg["kf"] = nc.dram_tensor("dbg_kf", [P, NPAIR * 2 * S], F8, kind="ExternalOutput")
    with tile.TileContext(nc) as tc:
        emit(nc, tc, x, wq_d, wk_d, wv_d, wo_d, out_d, dbg=dbg)
    split_excess_waits(nc, compute_max_waits=COMPUTE_MAX_WAITS)
    return nc


_cache = {}


def _get_runner():
    """Persistent jitted 8-core runner (mirrors bass2jax.run_bass_via_pjrt's
    multi-core path, but reusable across calls so we can time executions)."""
    if "runner" in _cache:
        return _cache["runner"]
    import jax
    from jax.experimental.shard_map import shard_map
    from jax.sharding import Mesh, PartitionSpec
    from concourse import bass2jax

    bass2jax.install_neuronx_cc_hook()
    if "nc" not in _cache:
        _cache["nc"] = build_nc()
    nc = _cache["nc"]

    partition_name = nc.partition_id_tensor.name if nc.partition_id_tensor else None
    in_names, out_names, out_avals = [], [], []
    for alloc in nc.m.functions[0].allocations:
        if not isinstance(alloc, mybir.MemoryLocationSet):
            continue
        name = alloc.memorylocations[0].name
        if alloc.kind == "ExternalInput":
            if name != partition_name:
                in_names.append(name)
        elif alloc.kind == "ExternalOutput":
            out_names.append(name)
            out_avals.append(
                jax.core.ShapedArray(tuple(alloc.tensor_shape), mybir.dt.np(alloc.dtype))
            )
    n_params, n_outs = len(in_names), len(out_names)
    all_names = list(in_names) + list(out_names)
    if partition_name is not None:
        all_names.append(partition_name)
    all_names = tuple(all_names)

    def _body(*args):
        operands = list(args)
        if partition_name is not None:
            operands.append(bass2jax.partition_id_tensor())
        outs = bass2jax._bass_exec_p.bind(
            *operands,
            out_avals=tuple(out_avals),
            in_names=all_names,
            out_names=tuple(out_names),
            lowering_input_output_aliases=(),
            sim_require_finite=True,
            sim_require_nnan=True,
            nc=nc,
        )
        return tuple(outs)

    devices = jax.devices()[:8]
    mesh = Mesh(np.asarray(devices), ("core",))
    in_specs = (PartitionSpec("core"),) * (n_params + n_outs)
    out_specs = (PartitionSpec("core"),) * n_outs
    donate = tuple(range(n_params, n_params + n_outs))
    sharded = jax.jit(
        shard_map(_body, mesh=mesh, in_specs=in_specs, out_specs=out_specs, check_rep=False),
        donate_argnums=donate,
        keep_unused=True,
    )
    _cache["runner"] = (sharded, in_names, out_names, out_avals, mesh)
    return _cache["runner"]


def run_on_cores(in_maps):
    """Run the kernel on 8 cores; returns list of per-core output dicts."""
    sharded, in_names, out_names, out_avals, mesh = _get_runner()
    concat_in = [
        np.concatenate([np.asarray(in_maps[c][name]) for c in range(8)], axis=0)
        for name in in_names
    ]
    concat_zeros = [
        np.zeros((8 * a.shape[0], *a.shape[1:]), a.dtype) for a in out_avals
    ]
    out_arrs = sharded(*concat_in, *concat_zeros)
    return [
        {
            name: np.asarray(out_arrs[i]).reshape(8, *out_avals[i].shape)[c]
            for i, name in enumerate(out_names)
        }
        for c in range(8)
    ]


def make_in_maps(residual, W_Q, W_K, W_V, W_O):
    in_maps = []
    for core in range(8):
        b, g = core // 2, core % 2
        sl = slice(8 * g, 8 * (g + 1))
        blob = np.concatenate(
            [
                np.asarray(residual[b], dtype=np.float32).reshape(S, D),
                np.asarray(W_Q[sl], dtype=np.float32).reshape(NH * H, D),
                np.asarray(W_K[sl], dtype=np.float32).reshape(NH * H, D),
                np.asarray(W_V[sl], dtype=np.float32).reshape(NH * H, D),
                np.asarray(W_O[sl], dtype=np.float32).reshape(NH * H, D),
            ],
            axis=0,
        )
        in_maps.append({"blob": np.ascontiguousarray(blob)})
    return in_maps


def kernel(residual, W_Q, W_K, W_V, W_O):
    residual = np.asarray(residual)
    in_maps = make_in_maps(residual, W_Q, W_K, W_V, W_O)
    results = run_on_cores(in_maps)
    B = residual.shape[0]
    out = np.zeros((B, S, D), np.float32)
    for core in range(8):
        b = core // 2
        out[b] += results[core]["out"]
    return out


if __name__ == "__main__":
    rng = np.random.default_rng(0)
    residual = rng.standard_normal((4, S, D)).astype(np.float32)
    W = [0.02 * rng.standard_normal((16, H, D)).astype(np.float32) for _ in range(4)]
    out = kernel(residual, *W)
    print("kernel ran, out shape", out.shape, "finite:", np.isfinite(out).all())

